# revision 8
# baseline (speedup 1.0000x reference)
"""Trainium2 Bass kernel for nn_MultiHeadAttention_2250562863251.

Key algebraic insight: the reference einsum 'mbhi,nbhj->mnbh' contracts i and j
independently, so scores[m,n,b,h] = (sum_i q[m,b,h,i]) * (sum_j k[n,b,h,j]) --
a rank-1 outer product of per-head row-sums. Full Q/K projections are never
needed; only queries @ (per-head-summed Wq) [E,16], computed on host (tiny).

Sharding: 8 cores = 2 (batch) x 4 (head-groups of 4 heads). SPMD program via
run_bass_kernel_spmd; host shards inputs / gathers + reduces outputs.

v2 restructure (from trace analysis of the v1 kernel: DVE 135us / ACT 133us /
PE 117us over a 197us span, ACT paying 352cy fixed per 512-col exp op and DVE
running stt at fp32 1x mode):
  - padding mask folded into V: host zeroes masked columns of valuesT and a
    tiny ones-mask DMA zeroes the denominator ones-column rows, so exp needs
    NO per-partition bias -> ONE fused ACTIVATE per (m-tile, head) strip
    (16 exp ops instead of 202).
  - scores built by DVE stt in all-16-bit operands (fp16 qs broadcast, fp16
    beta / beta+tri broadcast) -> 2x_1P mode (2 elem/cycle/lane).
  - causal tri handled by a combined beta+tri tile bbt[p,h,m] = beta[h,m] +
    TRI[p, m%128] (the 128-col triangle repeats, each column block serving as
    the diagonal in1 exactly for its chunk) -- replaces the old 5MB btri
    stream with a 2MB resident tile.
  - softmax divide fused with the PSUM evacuation: ACT copies pool PSUM ->
    SBUF bf16 (denominator row rides along), DVE divides at 2x from SBUF.
  - out-proj results copied PSUM->SBUF (DMA cannot read PSUM) split between
    DVE and ACT, then DMA'd out.
  - epilogues software-pipelined one m-tile behind the score loop so the
    reciprocal DMA round-trip never stalls an engine stream.
"""
import sys

for _p in ("/opt/trn_rl_repo", "/root/.axon_site/_ro/trn_rl_repo"):
    if _p not in sys.path:
        sys.path.append(_p)

import numpy as np
import ml_dtypes

import concourse.bass as bass
import concourse.mybir as mybir
import concourse.tile as tile
from concourse import bacc
from concourse.bass_utils import run_bass_kernel_spmd

# Problem shapes (hardcoded per contract)
M = 2048   # query positions
N = 2048   # key positions
B = 2
E = 1024
H = 16
DH = 64        # head dim
HL = 4         # heads per core
KL = HL * DH   # 256 local pooled dims
NEG = -1000.0
P = 128
NK = N // P    # 16 n-chunks
T = 4          # m-tiles of 512
MT = 512
NCORES = 8
OFFD = (0, 128, 384, 768)   # diag-chunk packing offsets (widths 128/256/384/512)

f32 = mybir.dt.float32
f16 = mybir.dt.float16
bf16 = mybir.dt.bfloat16

_CACHE = {}


def _build_program():
    if "nc" in _CACHE:
        return _CACHE["nc"]
    nc = bacc.Bacc("TRN2", target_bir_lowering=False, debug=False,
                   num_devices=NCORES)

    vt_d = nc.declare_dram_parameter("vt", [P, 4, (E // P) * MT], bf16, isOutput=False)
    wvlt_d = nc.declare_dram_parameter("wvlt", [P, (E // P) * KL], bf16, isOutput=False)
    wolt_d = nc.declare_dram_parameter("wolt", [P, (KL // P) * E], bf16, isOutput=False)
    qsl_d = nc.declare_dram_parameter("qsl", [HL, M], f16, isOutput=False)
    bbl_d = nc.declare_dram_parameter("bbl", [HL, M], f16, isOutput=False)
    bbt_d = nc.declare_dram_parameter("bbt", [P, HL * M], f16, isOutput=False)
    cd_d = nc.declare_dram_parameter("cd", [P, NK * HL], f32, isOutput=False)
    ones_d = nc.declare_dram_parameter("onesm", [P, NK * HL], bf16, isOutput=False)
    # blocked output: [ob, t, 128, 512] -> host reassembles to [E, M]
    outp_d = nc.declare_dram_parameter("outp", [E // P, T, P, MT], f32,
                                       isOutput=True)

    with tile.TileContext(nc) as tc:
        with (
            tc.tile_pool(name="const", bufs=1) as const,
            tc.tile_pool(name="vstream", bufs=2) as vstream,
            tc.tile_pool(name="scpool", bufs=2) as scpool,
            tc.tile_pool(name="etpool", bufs=2) as etpool,
            tc.tile_pool(name="rspool", bufs=1) as rspool,
            tc.tile_pool(name="ptn", bufs=2) as ptn,
            tc.tile_pool(name="small", bufs=4) as small,
            tc.tile_pool(name="opool", bufs=3) as opool,
            tc.tile_pool(name="dpool", bufs=4, space="DRAM") as dpool,
            tc.tile_pool(name="ps_v", bufs=2, space="PSUM") as ps_v,
            tc.tile_pool(name="ps_pool", bufs=1, space="PSUM") as ps_pool,
        ):
            # ---- resident constants ----
            wvlt_sb = const.tile([P, E // P, KL], bf16)
            nc.sync.dma_start(wvlt_sb[:], wvlt_d.rearrange("p (ek d) -> p ek d", ek=E // P))
            cd_sb = const.tile([P, NK, HL], f32)
            nc.sync.dma_start(cd_sb[:], cd_d.rearrange("p (k f) -> p k f", k=NK))
            qsb = const.tile([P, HL, M], f16)
            nc.sync.dma_start(qsb[:], qsl_d[None, :, :].to_broadcast([P, HL, M]))
            bb = const.tile([P, HL, M], f16)
            nc.sync.dma_start(bb[:], bbl_d[None, :, :].to_broadcast([P, HL, M]))
            bbt = const.tile([P, HL, M], f16)
            nc.sync.dma_start(bbt[:], bbt_d.rearrange("p (h m) -> p h m", h=HL))
            wolt_sb = const.tile([P, KL // P, E], bf16)
            nc.sync.dma_start(wolt_sb[:], wolt_d.rearrange("p (kb o) -> p kb o", kb=KL // P))

            # v_sb[:, k, h*65 : h*65+64] = v for head h, chunk k; col 64 =
            # 1.0 where key is live, 0.0 where padded (denominator mask).
            v_sb = const.tile([P, NK, HL * (DH + 1)], bf16)
            nc.sync.dma_start(
                v_sb.rearrange("p k (h x) -> p k h x", x=DH + 1)[:, :, :, DH:DH + 1],
                ones_d.rearrange("p (k h x) -> p k h x", k=NK, x=1))

            # ---- stage 1: v projection (vt streamed per n-quarter) ----
            # quarters run DESCENDING: stage-2's k-loop consumes v_sb[15] first
            for q in range(3, -1, -1):
                vt_sb = vstream.tile([P, E // P, MT], bf16, tag="vt")
                nc.sync.dma_start(
                    vt_sb[:], vt_d[:, q].rearrange("p (ek n) -> p ek n", ek=E // P))
                for nk_r in range(3, -1, -1):
                    k = q * 4 + nk_r
                    vps = ps_v.tile([P, KL], f32, tag="vps")
                    for ek in range(E // P):
                        nc.tensor.matmul(
                            vps[:],
                            vt_sb[:, ek, nk_r * P:(nk_r + 1) * P],
                            wvlt_sb[:, ek, :],
                            start=(ek == 0),
                            stop=(ek == E // P - 1),
                        )
                    nc.vector.tensor_copy(
                        out=v_sb[:, k].rearrange("p (h x) -> p h x", x=DH + 1)[:, :, 0:DH],
                        in_=vps.rearrange("p (h x) -> p h x", x=DH),
                    )

            # ---- stage 2: scores / softmax / attention / out-proj ----
            # 16 units u = 4t + h. Score strips per unit: full chunks k =
            # NK-1 .. 4t+4 at offset (NK-1-k)*512, then 4 diag chunks packed
            # at WF + OFFD[pos]. One fused exp per unit. Pool-PSUM is
            # evacuated (ACT copy, bf16) one unit later; the reciprocal
            # chain + divide + out-proj for tile t runs early in tile t+1.
            pool_ps_l = [None] * 16   # live pool psum tiles by unit
            pool_sb_l = [None] * 16   # evacuated pool tiles by unit
            rdall_l = [None] * T
            rdall2_l = [None] * T

            def emit_unit(u):
                t, h = u // 4, u % 4
                ts = t * MT
                WF = (12 - 4 * t) * MT
                Wtot = WF + 1280
                sc = scpool.tile([P, Wtot], f16, tag="sc")
                for k in range(NK - 1, 4 * t + 3, -1):
                    off = (NK - 1 - k) * MT
                    nc.vector.scalar_tensor_tensor(
                        out=sc[:, off:off + MT],
                        in0=qsb[:, h, ts:ts + MT],
                        scalar=cd_sb[:, k, h:h + 1],
                        in1=bb[:, h, ts:ts + MT],
                        op0=mybir.AluOpType.mult,
                        op1=mybir.AluOpType.add,
                    )
                for pos in range(3, -1, -1):
                    k = 4 * t + pos
                    ob = WF + OFFD[pos]
                    lw = pos * P          # fully-live cols of this diag chunk
                    if lw:
                        nc.vector.scalar_tensor_tensor(
                            out=sc[:, ob:ob + lw],
                            in0=qsb[:, h, ts:ts + lw],
                            scalar=cd_sb[:, k, h:h + 1],
                            in1=bb[:, h, ts:ts + lw],
                            op0=mybir.AluOpType.mult,
                            op1=mybir.AluOpType.add,
                        )
                    nc.vector.scalar_tensor_tensor(
                        out=sc[:, ob + lw:ob + lw + P],
                        in0=qsb[:, h, ts + lw:ts + lw + P],
                        scalar=cd_sb[:, k, h:h + 1],
                        in1=bbt[:, h, ts + lw:ts + lw + P],
                        op0=mybir.AluOpType.mult,
                        op1=mybir.AluOpType.add,
                    )
                et = etpool.tile([P, Wtot], bf16, tag="et")
                nc.scalar.activation(et[:], sc[:],
                                     mybir.ActivationFunctionType.Exp)
                pool_ps = ps_pool.tile([DH + 1, MT], f32, tag=f"pool{h}")
                pool_ps_l[u] = pool_ps
                hs = slice(h * (DH + 1), (h + 1) * (DH + 1))
                for k in range(NK - 1, 4 * t + 3, -1):
                    off = (NK - 1 - k) * MT
                    nc.tensor.matmul(
                        pool_ps[:],
                        v_sb[:, k, hs],
                        et[:, off:off + MT],
                        start=(k == NK - 1),
                        stop=False,
                    )
                for pos in range(3, -1, -1):
                    k = 4 * t + pos
                    W = (pos + 1) * P
                    nc.tensor.matmul(
                        pool_ps[:, 0:W],
                        v_sb[:, k, hs],
                        et[:, WF + OFFD[pos]:WF + OFFD[pos] + W],
                        start=(t == 3 and pos == 3),
                        stop=(pos == 0),
                    )

            def emit_evac(u):
                # evacuate pool PSUM -> SBUF bf16 (ACT); denominator row to
                # DRAM for the partition-spread reciprocal.
                t, h = u // 4, u % 4
                pool_sb = rspool.tile([DH + 1, MT], f16, tag=f"pool_sb{u % 8}")
                nc.scalar.copy(pool_sb[:], pool_ps_l[u][:])
                pool_sb_l[u] = pool_sb
                if h == 0:
                    rdall_l[t] = dpool.tile([HL, MT], f16, tag=f"rd{t % 2}",
                                            name=f"rdall{t}")
                nc.sync.dma_start(rdall_l[t][h:h + 1, :], pool_sb[DH:DH + 1, :])

            def emit_epilogue(t):
                # reciprocal of the 4 denominator rows, partition-spread as
                # [128, 16] so one DVE op covers the m-tile; then divide,
                # head pair-merge, out-projection, PSUM evacuation, DMA out.
                rsg = small.tile([P, HL * MT // P], f16, tag="rsg")
                nc.sync.dma_start(
                    rsg[:], rdall_l[t].rearrange("a (b x) -> (a b) x", x=HL * MT // P))
                rsgr = small.tile([P, HL * MT // P], f16, tag="rsgr")
                with nc.allow_low_precision(reason="per-(m,h) softmax scale; fp16 ok"):
                    nc.vector.reciprocal(out=rsgr[:], in_=rsg[:])
                rdall2 = dpool.tile([HL, MT], f16, tag=f"rd2{t % 2}")
                nc.sync.dma_start(
                    rdall2.rearrange("a (b x) -> (a b) x", x=HL * MT // P), rsgr[:])
                pTn = ptn.tile([DH, HL, MT], bf16, tag="ptn")
                for h in range(HL):
                    rsb = small.tile([DH, MT], f16, tag=f"rsb{h % 2}")
                    nc.sync.dma_start(
                        rsb[:], rdall2[h][None, :].to_broadcast([DH, MT]))
                    nc.vector.tensor_mul(
                        out=pTn[:, h, :],
                        in0=pool_sb_l[4 * t + h][0:DH, :],
                        in1=rsb[:],
                    )
                # pair heads into 128 partitions for K=128 out-proj matmuls
                pTn2 = ptn.tile([P, KL // P, MT], bf16, tag="ptn2")
                for kb in range(KL // P):
                    nc.sync.dma_start(pTn2[0:DH, kb], pTn[:, 2 * kb, :])
                    nc.sync.dma_start(pTn2[DH:P, kb], pTn[:, 2 * kb + 1, :])
                for ob in range(E // P):
                    ops = ps_v.tile([P, MT], f32, tag="ops")
                    for kb in range(KL // P):
                        nc.tensor.matmul(
                            ops[:],
                            wolt_sb[:, kb, ob * P:(ob + 1) * P],
                            pTn2[:, kb, :],
                            start=(kb == 0),
                            stop=(kb == KL // P - 1),
                        )
                    osb = opool.tile([P, MT], f32, tag="osb")
                    if ob % 8 < 5:
                        nc.vector.tensor_copy(out=osb[:], in_=ops[:])
                    else:
                        nc.scalar.copy(osb[:], ops[:])
                    nc.sync.dma_start(outp_d[ob, t], osb[:])

            for u in range(17):
                if u < 16:
                    emit_unit(u)
                if u >= 1:
                    emit_evac(u - 1)
                if u >= 6 and u % 4 == 2:
                    emit_epilogue(u // 4 - 1)
            emit_epilogue(3)

    nc.compile()
    _CACHE["nc"] = nc
    return nc


def _host_prep(queries, keys, values, Wq, bq, Wk, bk, Wv, bv, Wo, bo, in_mask):
    """Host-side prep. Returns (in_maps, fixup, extras)."""
    qs = np.einsum("mbe,he->mbh", queries, Wq.reshape(H, DH, E).sum(1),
                   dtype=np.float32) + bq.reshape(H, DH).sum(1)
    ks = np.einsum("nbe,he->nbh", keys, Wk.reshape(H, DH, E).sum(1),
                   dtype=np.float32) + bk.reshape(H, DH).sum(1)
    # device multiplies the fp16-rounded qs; compute beta from the same values
    qs16 = qs.astype(np.float16)
    qsf = qs16.astype(np.float32)

    mask3 = in_mask[:, :, None]
    cp = np.where(mask3, 0.0, ks).astype(np.float32)          # [n, b, H]

    cmax = np.where(mask3, -np.inf, ks)
    cmax = np.maximum.accumulate(cmax[::-1], axis=0)[::-1]    # suffix max, n>=m
    cmin = np.where(mask3, np.inf, ks)
    cmin = np.minimum.accumulate(cmin[::-1], axis=0)[::-1]
    nonempty = np.maximum.accumulate((~in_mask)[::-1], axis=0)[::-1]  # [n, b]

    with np.errstate(invalid="ignore"):
        A = np.where(qsf >= 0, qsf * cmax, qsf * cmin)        # [m, b, H]
    A = np.where(nonempty[:, :, None], A, -np.inf)
    fixup_rows = np.any(~(A > -70.0), axis=2)                 # [m, b] (nan-safe)
    beta = np.where(np.isfinite(A), -A, 1e4)
    beta = np.where(fixup_rows[:, :, None], -1e4, beta)
    beta = beta.astype(np.float32)

    in_maps = []
    def pmajor(a, p=P):
        """[X*p, Y] -> [p, X*Y]: partition-major packing for 1-run-per-
        partition DMA loads matching 'p (x y) -> p x y' device views."""
        X = a.shape[0] // p
        return np.ascontiguousarray(
            a.reshape(X, p, a.shape[1]).transpose(1, 0, 2).reshape(p, -1))

    def pack_vt(vT):
        # [E, N] -> [P, 4, (E//P)*MT]: quarter-major, then ek-major
        a = vT.reshape(E // P, P, 4, MT)          # [ek, p, q, mt]
        return np.ascontiguousarray(
            a.transpose(1, 2, 0, 3).reshape(P, 4, (E // P) * MT))

    # zero masked key columns of v^T: their pooled contribution must vanish
    vt_by_b = []
    for bi in range(B):
        vT = values[:, bi, :].T.copy()
        vT[:, in_mask[:, bi]] = 0.0
        vt_by_b.append(pack_vt(vT.astype(ml_dtypes.bfloat16)))

    # ones-column mask [P, NK, HL]: 1.0 for live keys, 0.0 for padded
    live = (~in_mask).astype(np.float32)                      # [n, b]
    onesm_by_b = [
        np.ascontiguousarray(np.broadcast_to(
            live[:, bi].reshape(NK, P, 1).transpose(1, 0, 2), (P, NK, HL))
        ).reshape(P, NK * HL).astype(ml_dtypes.bfloat16)
        for bi in range(B)]

    # TRI[p, j]: -4000 where p < j (within a 128-col block) -- serves every
    # diagonal chunk since columns m in block k pair with chunk k partitions
    TRI = np.where(np.arange(P)[:, None] < np.arange(P)[None, :], -4000.0,
                   0.0).astype(np.float32)                    # [p, j]

    for c in range(NCORES):
        bi, hg = c // 4, c % 4
        lh = slice(hg * HL, (hg + 1) * HL)
        ds = slice(hg * KL, (hg + 1) * KL)
        beta_lh = np.ascontiguousarray(beta[:, bi, lh].T)     # [HL, M]
        # bbt[p, h, m] = beta[h, m] + TRI[p, m % 128]
        bbt_full = (beta_lh[None, :, :]
                    + np.tile(TRI, (1, NK)).reshape(P, 1, M))
        in_maps.append({
            "vt": vt_by_b[bi],
            "wvlt": pmajor(Wv[ds, :].T.astype(ml_dtypes.bfloat16)),
            "wolt": pmajor(Wo[:, ds].T.astype(ml_dtypes.bfloat16)),
            "qsl": np.ascontiguousarray(qs16[:, bi, lh].T),
            "bbl": beta_lh.astype(np.float16),
            "bbt": np.ascontiguousarray(
                bbt_full.reshape(P, HL * M)).astype(np.float16),
            "cd": pmajor(np.ascontiguousarray(cp[:, bi, lh])),
            "onesm": onesm_by_b[bi],
        })
    return in_maps, fixup_rows, (qsf, ks)


def _fixup_row(out, m, bi, qs, ks, values, Wv, bv, Wo, bo, in_mask):
    """Exact numpy recompute of one output row (degenerate / extreme rows)."""
    pot = qs[m, bi, :][None, :] * ks[:, bi, :]                # [n, H]
    pot = np.where(in_mask[:, bi][:, None], NEG, pot)
    causal = np.arange(N) < m                                 # mask n < m
    pot = np.where(causal[:, None], NEG, pot)
    pot = pot - pot.max(axis=0, keepdims=True)
    w = np.exp(pot)
    w = w / w.sum(axis=0, keepdims=True)                      # [n, H]
    v = (values[:, bi, :] @ Wv.T + bv).reshape(N, H, DH)
    pooled = np.einsum("nh,nhd->hd", w, v).reshape(E)
    out[m, bi, :] = pooled @ Wo.T + bo


def kernel(queries, keys, values, Wq, bq, Wk, bk, Wv, bv, Wo, bo, in_mask,
           _trace=False):
    args = (queries, keys, values, Wq, bq, Wk, bk, Wv, bv, Wo, bo)
    args = tuple(np.asarray(a, np.float32) for a in args)
    in_mask = np.asarray(in_mask, bool)
    (queries, keys, values, Wq, bq, Wk, bk, Wv, bv, Wo, bo) = args

    nc = _build_program()
    in_maps, fixup_rows, (qs, ks) = _host_prep(
        queries, keys, values, Wq, bq, Wk, bk, Wv, bv, Wo, bo, in_mask)

    res = run_bass_kernel_spmd(nc, in_maps, list(range(NCORES)), trace=_trace)
    results = res.results

    out = np.zeros((M, B, E), np.float32)
    for c in range(NCORES):
        bi = c // 4
        blk = np.asarray(results[c]["outp"], np.float32)   # [8, 4, 128, 512]
        outT = blk.transpose(0, 2, 1, 3).reshape(E, M)
        out[:, bi, :] += outT.T
    out += (bo + bv @ Wo.T)[None, None, :]

    for m, bi in zip(*np.nonzero(fixup_rows)):
        _fixup_row(out, m, bi, qs, ks, values, Wv, bv, Wo, bo, in_mask)

    if _trace:
        return out, res
    return out
